# revision 23
# baseline (speedup 1.0000x reference)
"""Trainium2 Bass kernel for int8-valued Conv2d(128->256, 3x3, pad 1) + BN-add +
shift requant + clip + uint8 cast, over x[32,128,56,56].

Strategy: data-parallel over batch across 8 NeuronCores (4 images/core).
Per core, the conv runs as 9 PSUM-accumulated bf16 matmuls (one per 3x3 tap)
with Cin=128 on the partition axis. int8-valued data is exact in bf16, and all
fp32 accumulations stay below 2^24, so the matmul path is integer-exact.
Input is laid out zero-padded to 58x58 in SBUF so each tap's rhs is a plain
column-shifted window. Output rows are produced in chunks of 8 rows
(N = 8*58 = 464 <= 512 fp32 PSUM bank columns).

Requant (matches reference op order): ACT adds per-channel t (fp32, integral,
out int32), DVE arithmetic-shifts right by s (int32->int32; bitwise ops cannot
cast), DVE clamps to [act_min, act_max] (fp32 ALU pair, writes uint8). Every
dtype conversion happens on integral values, so rounding-mode differences
between sim and HW are moot.
"""

import numpy as np
import ml_dtypes
from contextlib import ExitStack

import concourse.bass as bass  # noqa: F401  (registers engine types)
import concourse.mybir as mybir
import concourse.tile as tile
from concourse import bacc
from concourse.bass_utils import run_bass_kernel_spmd

# Problem constants (hardcoded per contract)
N_CORES = 8
B = 32
B_LOC = B // N_CORES          # 4 images per core
P = 128                       # Cin = partition dim
H = W = 56
Hp = Wp = 58                  # padded
IMG = Hp * Wp                 # 3364 padded pixels per image
NB = B_LOC * IMG + 2          # + leading/trailing guard column
COUT = 256
G = COUT // P                 # 2 Cout halves
ROWS_PER_CH = 8
N_CH = H // ROWS_PER_CH       # 7 chunks per image
NFREE = ROWS_PER_CH * Wp      # 464 columns per matmul

_cache = {}


def _build(shift: int):
    """Build + compile the per-core Bass program. Same NEFF on all 8 cores."""
    nc = bacc.Bacc("TRN2", target_bir_lowering=False, debug=False,
                   num_devices=N_CORES)

    xs = nc.dram_tensor("xs", [B_LOC, P, H, W], mybir.dt.int8, kind="ExternalInput")
    wt = nc.dram_tensor("wt", [P, 9 * COUT], mybir.dt.bfloat16, kind="ExternalInput")
    tb = nc.dram_tensor("tb", [P, G], mybir.dt.float32, kind="ExternalInput")
    lo = nc.dram_tensor("lo", [P, G], mybir.dt.float32, kind="ExternalInput")
    hi = nc.dram_tensor("hi", [P, G], mybir.dt.float32, kind="ExternalInput")
    ys = nc.dram_tensor("ys", [B_LOC, COUT, H, W], mybir.dt.uint8, kind="ExternalOutput")

    with tile.TileContext(nc) as tc, ExitStack() as ctx:
        wpool = ctx.enter_context(tc.tile_pool(name="wpool", bufs=1))
        cpool = ctx.enter_context(tc.tile_pool(name="cpool", bufs=1))
        xspool = ctx.enter_context(tc.tile_pool(name="xspool", bufs=3))
        xppool = ctx.enter_context(tc.tile_pool(name="xppool", bufs=1))
        pspool = ctx.enter_context(tc.tile_pool(name="pspool", bufs=6, space="PSUM"))
        i1pool = ctx.enter_context(tc.tile_pool(name="i1pool", bufs=2))
        i2pool = ctx.enter_context(tc.tile_pool(name="i2pool", bufs=2))
        opool = ctx.enter_context(tc.tile_pool(name="opool", bufs=3))

        wt_s = wpool.tile([P, 9 * COUT], mybir.dt.bfloat16)
        tb_s = cpool.tile([P, G], mybir.dt.float32)
        lo_s = cpool.tile([P, G], mybir.dt.float32)
        hi_s = cpool.tile([P, G], mybir.dt.float32)
        xpad = xppool.tile([P, NB], mybir.dt.bfloat16)
        nc.vector.memset(xpad[:, 0:1], 0.0)
        nc.vector.memset(xpad[:, NB - 1:NB], 0.0)
        inners = []
        for img in range(B_LOC):
            base = 1 + img * IMG
            # pad row 0 (+ row 1 col 0)
            nc.vector.memset(xpad[:, base:base + Wp + 1], 0.0)
            # pad row Hp-1
            nc.vector.memset(xpad[:, base + (Hp - 1) * Wp: base + IMG], 0.0)
            inner = xpad[:, base + Wp: base + Wp + H * Wp].rearrange(
                "p (h w) -> p h w", w=Wp)
            nc.vector.memset(inner[:, :, Wp - 1:Wp], 0.0)   # col Wp-1, rows 1..H
            nc.vector.memset(inner[:, 1:, 0:1], 0.0)        # col 0, rows 2..H
            inners.append(inner)

        # Sub-image load/cast granularity so the first matmuls start sooner;
        # critical-path deps (x rows 0.., then tap0/g0 weights) issue first.
        def load_rows(img, r0, nrows):
            xstage = xspool.tile([P, (H // 2) * W], mybir.dt.int8, name="xstage")
            st = xstage[:, :nrows * W]
            nc.sync.dma_start(
                st, xs.ap()[img, :, r0:r0 + nrows, :].rearrange("c h w -> c (h w)"))
            # cast int8 -> bf16 into padded interior (exact: |x| <= 256)
            nc.scalar.copy(inners[img][:, r0:r0 + nrows, 1:1 + W],
                           st.rearrange("p (h w) -> p h w", w=W))

        QR = H // 4
        load_rows(0, 0, QR)
        nc.sync.dma_start(wt_s[:, :P], wt.ap()[:, :P])            # tap0, g=0
        load_rows(0, QR, QR)
        nc.sync.dma_start(wt_s[:, P:9 * P], wt.ap()[:, P:9 * P])  # taps 1-8, g=0
        nc.sync.dma_start(tb_s[:], tb.ap())
        nc.sync.dma_start(lo_s[:], lo.ap())
        nc.sync.dma_start(hi_s[:], hi.ap())
        load_rows(0, 2 * QR, QR)
        load_rows(0, 3 * QR, QR)
        load_rows(1, 0, H // 2)                                   # needed ~25us
        nc.sync.dma_start(wt_s[:, 9 * P:], wt.ap()[:, 9 * P:])    # g=1, needed ~38us
        load_rows(1, H // 2, H // 2)
        for img in range(2, B_LOC):
            load_rows(img, 0, H // 2)
            load_rows(img, H // 2, H // 2)

        for img in range(B_LOC):
            for g in range(G):
                base = 1 + img * IMG
                for ch in range(N_CH):
                    hp0 = 1 + ch * ROWS_PER_CH
                    psumt = pspool.tile([P, NFREE], mybir.dt.float32)
                    for tap in range(9):
                        dh, dw = tap // 3, tap % 3
                        rs = base + (hp0 + dh - 1) * Wp + (dw - 1)
                        nc.tensor.matmul(
                            psumt[:],
                            lhsT=wt_s[:, (g * 9 + tap) * P:(g * 9 + tap + 1) * P],
                            rhs=xpad[:, rs:rs + NFREE],
                            start=(tap == 0),
                            stop=(tap == 8),
                        )
                    # Last chunk: split the requant chain in two so the
                    # post-matmul tail is shorter; otherwise one pass.
                    last = (img == B_LOC - 1 and g == G - 1 and ch == N_CH - 1)
                    for r0, rn in ([(0, 4), (4, 4)] if last
                                   else [(0, ROWS_PER_CH)]):
                        cs, cn = r0 * Wp, rn * Wp
                        it = i1pool.tile([P, NFREE], mybir.dt.int32,
                                         name="it")[:, :cn]
                        nc.scalar.activation(it, psumt[:, cs:cs + cn],
                                             mybir.ActivationFunctionType.Identity,
                                             bias=tb_s[:, g:g + 1], scale=1.0)
                        ct = i2pool.tile([P, NFREE], mybir.dt.int32,
                                         name="ct")[:, :cn]
                        nc.vector.tensor_scalar(ct, it, shift, None,
                                                mybir.AluOpType.arith_shift_right)
                        # clamp + compact pad columns: strided read, tight write
                        ot = opool.tile([P, ROWS_PER_CH * W], mybir.dt.uint8,
                                        name="ot")[:, :rn * W]
                        csrc = ct.rearrange("p (r w) -> p r w", w=Wp)[:, :, 1:1 + W]
                        nc.vector.tensor_scalar(
                            ot.rearrange("p (r w) -> p r w", w=W), csrc,
                            lo_s[:, g:g + 1], hi_s[:, g:g + 1],
                            mybir.AluOpType.max, mybir.AluOpType.min)
                        # contiguous per partition on both sides
                        nc.sync.dma_start(
                            ys.ap()[img, g * P:(g + 1) * P,
                                    hp0 - 1 + r0:hp0 - 1 + r0 + rn, :]
                            .rearrange("c h w -> c (h w)"),
                            ot)

    nc.compile()
    return nc


def _pack_inputs(x, weight, t, n, act_min, act_max):
    x = np.asarray(x)
    weight = np.asarray(weight)
    t = np.asarray(t).reshape(COUT)
    n = np.asarray(n).reshape(COUT)
    act_min = np.asarray(act_min).reshape(COUT)
    act_max = np.asarray(act_max).reshape(COUT)

    assert x.shape == (B, P, H, W) and weight.shape == (COUT, P, 3, 3)
    nval = int(n[0])
    assert np.all(n == nval) and nval <= 0, "non-uniform/positive BN shift unsupported"
    shift = -nval
    assert np.all(act_min >= 0) and np.all(act_max <= 255), \
        "act range must fit uint8 (pure_positive path)"
    # int8 shipping + bf16/fp32 exactness preconditions
    assert x.min() >= -128 and x.max() <= 127
    assert np.abs(weight).max(initial=0) <= 256

    # lhsT pack, g-major: wt[ci, (g*9+tap)*P + co] = weight[g*P+co, ci, kh, kw]
    wr = weight.reshape(G, P, P, 9)            # [g, co, ci, tap]
    wr = wr.transpose(2, 0, 3, 1)              # [ci, g, tap, co]
    wt_np = np.ascontiguousarray(wr.reshape(P, 9 * COUT)).astype(ml_dtypes.bfloat16)

    tb_np = np.ascontiguousarray(t.reshape(G, P).T).astype(np.float32)
    lo_np = np.ascontiguousarray(act_min.reshape(G, P).T).astype(np.float32)
    hi_np = np.ascontiguousarray(act_max.reshape(G, P).T).astype(np.float32)
    return x, wt_np, tb_np, lo_np, hi_np, shift


def kernel(x, weight, t, n, act_min, act_max):
    x, wt_np, tb_np, lo_np, hi_np, shift = _pack_inputs(
        x, weight, t, n, act_min, act_max)

    if shift not in _cache:
        _cache[shift] = _build(shift)
    nc = _cache[shift]

    x8 = x.astype(np.int8)  # exact: setup guarantees int8-valued data
    in_maps = []
    for c in range(N_CORES):
        in_maps.append({
            "xs": np.ascontiguousarray(x8[c * B_LOC:(c + 1) * B_LOC]),
            "wt": wt_np,
            "tb": tb_np,
            "lo": lo_np,
            "hi": hi_np,
        })
    res = run_bass_kernel_spmd(nc, in_maps, core_ids=list(range(N_CORES)))
    out = np.concatenate([res.results[c]["ys"] for c in range(N_CORES)], axis=0)
    return out


# revision 25
# speedup vs baseline: 1.0056x; 1.0056x over previous
"""Trainium2 Bass kernel for int8-valued Conv2d(128->256, 3x3, pad 1) + BN-add +
shift requant + clip + uint8 cast, over x[32,128,56,56].

Strategy: data-parallel over batch across 8 NeuronCores (4 images/core).
Per core, the conv runs as 9 PSUM-accumulated bf16 matmuls (one per 3x3 tap)
with Cin=128 on the partition axis. int8-valued data is exact in bf16, and all
fp32 accumulations stay below 2^24, so the matmul path is integer-exact.
Input is laid out zero-padded to 58x58 in SBUF so each tap's rhs is a plain
column-shifted window. Output rows are produced in chunks of 8 rows
(N = 8*58 = 464 <= 512 fp32 PSUM bank columns).

Requant (matches reference op order): ACT adds per-channel t (fp32, integral,
out int32), DVE arithmetic-shifts right by s (int32->int32; bitwise ops cannot
cast), DVE clamps to [act_min, act_max] (fp32 ALU pair, writes uint8). Every
dtype conversion happens on integral values, so rounding-mode differences
between sim and HW are moot.
"""

import numpy as np
import ml_dtypes
from contextlib import ExitStack

import concourse.bass as bass  # noqa: F401  (registers engine types)
import concourse.mybir as mybir
import concourse.tile as tile
from concourse import bacc
from concourse.bass_utils import run_bass_kernel_spmd

# Problem constants (hardcoded per contract)
N_CORES = 8
B = 32
B_LOC = B // N_CORES          # 4 images per core
P = 128                       # Cin = partition dim
H = W = 56
Hp = Wp = 58                  # padded
IMG = Hp * Wp                 # 3364 padded pixels per image
NB = B_LOC * IMG + 2          # + leading/trailing guard column
COUT = 256
G = COUT // P                 # 2 Cout halves
ROWS_PER_CH = 8
N_CH = H // ROWS_PER_CH       # 7 chunks per image
NFREE = ROWS_PER_CH * Wp      # 464 columns per matmul

_cache = {}


def _build(shift: int):
    """Build + compile the per-core Bass program. Same NEFF on all 8 cores."""
    nc = bacc.Bacc("TRN2", target_bir_lowering=False, debug=False,
                   num_devices=N_CORES)

    xs = nc.dram_tensor("xs", [B_LOC, P, H, W], mybir.dt.int8, kind="ExternalInput")
    wt = nc.dram_tensor("wt", [P, 9 * COUT], mybir.dt.bfloat16, kind="ExternalInput")
    tb = nc.dram_tensor("tb", [P, G], mybir.dt.float32, kind="ExternalInput")
    lo = nc.dram_tensor("lo", [P, G], mybir.dt.float32, kind="ExternalInput")
    hi = nc.dram_tensor("hi", [P, G], mybir.dt.float32, kind="ExternalInput")
    ys = nc.dram_tensor("ys", [B_LOC, COUT, H, W], mybir.dt.uint8, kind="ExternalOutput")

    with tile.TileContext(nc) as tc, ExitStack() as ctx:
        wpool = ctx.enter_context(tc.tile_pool(name="wpool", bufs=1))
        cpool = ctx.enter_context(tc.tile_pool(name="cpool", bufs=1))
        xspool = ctx.enter_context(tc.tile_pool(name="xspool", bufs=3))
        xppool = ctx.enter_context(tc.tile_pool(name="xppool", bufs=1))
        pspool = ctx.enter_context(tc.tile_pool(name="pspool", bufs=6, space="PSUM"))
        i1pool = ctx.enter_context(tc.tile_pool(name="i1pool", bufs=2))
        i2pool = ctx.enter_context(tc.tile_pool(name="i2pool", bufs=2))
        opool = ctx.enter_context(tc.tile_pool(name="opool", bufs=3))

        wt_s = wpool.tile([P, 9 * COUT], mybir.dt.bfloat16)
        tb_s = cpool.tile([P, G], mybir.dt.float32)
        lo_s = cpool.tile([P, G], mybir.dt.float32)
        hi_s = cpool.tile([P, G], mybir.dt.float32)
        xpad = xppool.tile([P, NB], mybir.dt.bfloat16)

        # HAM pre-warm: the PE is idle ~7-11us at start while input DMAs run.
        # A stream of zero matmuls during that window gets the clock gate to
        # K=8/8 (2.4 GHz) before the first real matmul; no data deps.
        zpool = ctx.enter_context(tc.tile_pool(name="zpool", bufs=1))
        wupool = ctx.enter_context(tc.tile_pool(name="wupool", bufs=1,
                                                space="PSUM"))
        zs = zpool.tile([P, 384], mybir.dt.bfloat16)
        nc.vector.memset(zs[:], 0.0)
        wps = wupool.tile([P, 384], mybir.dt.float32)
        for _ in range(10):
            nc.tensor.matmul(wps[:], lhsT=zs[:, :P], rhs=zs[:],
                             start=True, stop=True)
        nc.vector.memset(xpad[:, 0:1], 0.0)
        nc.vector.memset(xpad[:, NB - 1:NB], 0.0)
        inners = []
        for img in range(B_LOC):
            base = 1 + img * IMG
            # pad row 0 (+ row 1 col 0)
            nc.vector.memset(xpad[:, base:base + Wp + 1], 0.0)
            # pad row Hp-1
            nc.vector.memset(xpad[:, base + (Hp - 1) * Wp: base + IMG], 0.0)
            inner = xpad[:, base + Wp: base + Wp + H * Wp].rearrange(
                "p (h w) -> p h w", w=Wp)
            nc.vector.memset(inner[:, :, Wp - 1:Wp], 0.0)   # col Wp-1, rows 1..H
            nc.vector.memset(inner[:, 1:, 0:1], 0.0)        # col 0, rows 2..H
            inners.append(inner)

        # Sub-image load/cast granularity so the first matmuls start sooner;
        # critical-path deps (x rows 0.., then tap0/g0 weights) issue first.
        def load_rows(img, r0, nrows):
            xstage = xspool.tile([P, (H // 2) * W], mybir.dt.int8, name="xstage")
            st = xstage[:, :nrows * W]
            nc.sync.dma_start(
                st, xs.ap()[img, :, r0:r0 + nrows, :].rearrange("c h w -> c (h w)"))
            # cast int8 -> bf16 into padded interior (exact: |x| <= 256)
            nc.scalar.copy(inners[img][:, r0:r0 + nrows, 1:1 + W],
                           st.rearrange("p (h w) -> p h w", w=W))

        QR = H // 4
        load_rows(0, 0, QR)
        nc.sync.dma_start(wt_s[:, :P], wt.ap()[:, :P])            # tap0, g=0
        nc.sync.dma_start(wt_s[:, P:9 * P], wt.ap()[:, P:9 * P])  # taps 1-8, g=0
        load_rows(0, QR, QR)
        nc.sync.dma_start(tb_s[:], tb.ap())
        nc.sync.dma_start(lo_s[:], lo.ap())
        nc.sync.dma_start(hi_s[:], hi.ap())
        load_rows(0, 2 * QR, QR)
        load_rows(0, 3 * QR, QR)
        load_rows(1, 0, H // 2)                                   # needed ~25us
        nc.sync.dma_start(wt_s[:, 9 * P:], wt.ap()[:, 9 * P:])    # g=1, needed ~38us
        load_rows(1, H // 2, H // 2)
        for img in range(2, B_LOC):
            load_rows(img, 0, H // 2)
            load_rows(img, H // 2, H // 2)

        for img in range(B_LOC):
            for g in range(G):
                base = 1 + img * IMG
                for ch in range(N_CH):
                    hp0 = 1 + ch * ROWS_PER_CH
                    psumt = pspool.tile([P, NFREE], mybir.dt.float32)
                    for tap in range(9):
                        dh, dw = tap // 3, tap % 3
                        rs = base + (hp0 + dh - 1) * Wp + (dw - 1)
                        nc.tensor.matmul(
                            psumt[:],
                            lhsT=wt_s[:, (g * 9 + tap) * P:(g * 9 + tap + 1) * P],
                            rhs=xpad[:, rs:rs + NFREE],
                            start=(tap == 0),
                            stop=(tap == 8),
                        )
                    # Last chunk: split the requant chain in two so the
                    # post-matmul tail is shorter; otherwise one pass.
                    last = (img == B_LOC - 1 and g == G - 1 and ch == N_CH - 1)
                    for r0, rn in ([(0, 4), (4, 4)] if last
                                   else [(0, ROWS_PER_CH)]):
                        cs, cn = r0 * Wp, rn * Wp
                        it = i1pool.tile([P, NFREE], mybir.dt.int32,
                                         name="it")[:, :cn]
                        nc.scalar.activation(it, psumt[:, cs:cs + cn],
                                             mybir.ActivationFunctionType.Identity,
                                             bias=tb_s[:, g:g + 1], scale=1.0)
                        ct = i2pool.tile([P, NFREE], mybir.dt.int32,
                                         name="ct")[:, :cn]
                        nc.vector.tensor_scalar(ct, it, shift, None,
                                                mybir.AluOpType.arith_shift_right)
                        # clamp + compact pad columns: strided read, tight write
                        ot = opool.tile([P, ROWS_PER_CH * W], mybir.dt.uint8,
                                        name="ot")[:, :rn * W]
                        csrc = ct.rearrange("p (r w) -> p r w", w=Wp)[:, :, 1:1 + W]
                        nc.vector.tensor_scalar(
                            ot.rearrange("p (r w) -> p r w", w=W), csrc,
                            lo_s[:, g:g + 1], hi_s[:, g:g + 1],
                            mybir.AluOpType.max, mybir.AluOpType.min)
                        # contiguous per partition on both sides
                        nc.sync.dma_start(
                            ys.ap()[img, g * P:(g + 1) * P,
                                    hp0 - 1 + r0:hp0 - 1 + r0 + rn, :]
                            .rearrange("c h w -> c (h w)"),
                            ot)

    nc.compile()
    return nc


def _pack_inputs(x, weight, t, n, act_min, act_max):
    x = np.asarray(x)
    weight = np.asarray(weight)
    t = np.asarray(t).reshape(COUT)
    n = np.asarray(n).reshape(COUT)
    act_min = np.asarray(act_min).reshape(COUT)
    act_max = np.asarray(act_max).reshape(COUT)

    assert x.shape == (B, P, H, W) and weight.shape == (COUT, P, 3, 3)
    nval = int(n[0])
    assert np.all(n == nval) and nval <= 0, "non-uniform/positive BN shift unsupported"
    shift = -nval
    assert np.all(act_min >= 0) and np.all(act_max <= 255), \
        "act range must fit uint8 (pure_positive path)"
    # int8 shipping + bf16/fp32 exactness preconditions
    assert x.min() >= -128 and x.max() <= 127
    assert np.abs(weight).max(initial=0) <= 256

    # lhsT pack, g-major: wt[ci, (g*9+tap)*P + co] = weight[g*P+co, ci, kh, kw]
    wr = weight.reshape(G, P, P, 9)            # [g, co, ci, tap]
    wr = wr.transpose(2, 0, 3, 1)              # [ci, g, tap, co]
    wt_np = np.ascontiguousarray(wr.reshape(P, 9 * COUT)).astype(ml_dtypes.bfloat16)

    tb_np = np.ascontiguousarray(t.reshape(G, P).T).astype(np.float32)
    lo_np = np.ascontiguousarray(act_min.reshape(G, P).T).astype(np.float32)
    hi_np = np.ascontiguousarray(act_max.reshape(G, P).T).astype(np.float32)
    return x, wt_np, tb_np, lo_np, hi_np, shift


def kernel(x, weight, t, n, act_min, act_max):
    x, wt_np, tb_np, lo_np, hi_np, shift = _pack_inputs(
        x, weight, t, n, act_min, act_max)

    if shift not in _cache:
        _cache[shift] = _build(shift)
    nc = _cache[shift]

    x8 = x.astype(np.int8)  # exact: setup guarantees int8-valued data
    in_maps = []
    for c in range(N_CORES):
        in_maps.append({
            "xs": np.ascontiguousarray(x8[c * B_LOC:(c + 1) * B_LOC]),
            "wt": wt_np,
            "tb": tb_np,
            "lo": lo_np,
            "hi": hi_np,
        })
    res = run_bass_kernel_spmd(nc, in_maps, core_ids=list(range(N_CORES)))
    out = np.concatenate([res.results[c]["ys"] for c in range(N_CORES)], axis=0)
    return out


# revision 27
# speedup vs baseline: 1.0271x; 1.0214x over previous
"""Trainium2 Bass kernel for int8-valued Conv2d(128->256, 3x3, pad 1) + BN-add +
shift requant + clip + uint8 cast, over x[32,128,56,56].

Strategy: data-parallel over batch across 8 NeuronCores (4 images/core).
Per core, the conv runs as 9 PSUM-accumulated bf16 matmuls (one per 3x3 tap)
with Cin=128 on the partition axis. int8-valued data is exact in bf16, and all
fp32 accumulations stay below 2^24, so the matmul path is integer-exact.
Input is laid out zero-padded to 58x58 in SBUF so each tap's rhs is a plain
column-shifted window. Output rows are produced in chunks of 8 rows
(N = 8*58 = 464 <= 512 fp32 PSUM bank columns).

Requant (matches reference op order): ACT adds per-channel t (fp32, integral,
out int32), DVE arithmetic-shifts right by s (int32->int32; bitwise ops cannot
cast), DVE clamps to [act_min, act_max] (fp32 ALU pair, writes uint8). Every
dtype conversion happens on integral values, so rounding-mode differences
between sim and HW are moot.
"""

import numpy as np
import ml_dtypes
from contextlib import ExitStack

import concourse.bass as bass  # noqa: F401  (registers engine types)
import concourse.mybir as mybir
import concourse.tile as tile
from concourse import bacc
from concourse.bass_utils import run_bass_kernel_spmd

# Problem constants (hardcoded per contract)
N_CORES = 8
B = 32
B_LOC = B // N_CORES          # 4 images per core
P = 128                       # Cin = partition dim
H = W = 56
Hp = Wp = 58                  # padded
IMG = Hp * Wp                 # 3364 padded pixels per image
NB = B_LOC * IMG + 2          # + leading/trailing guard column
COUT = 256
G = COUT // P                 # 2 Cout halves
ROWS_PER_CH = 8
N_CH = H // ROWS_PER_CH       # 7 chunks per image
NFREE = ROWS_PER_CH * Wp      # 464 columns per matmul

_cache = {}


def _build(shift: int):
    """Build + compile the per-core Bass program. Same NEFF on all 8 cores."""
    nc = bacc.Bacc("TRN2", target_bir_lowering=False, debug=False,
                   num_devices=N_CORES)

    xs = nc.dram_tensor("xs", [B_LOC, P, H, W], mybir.dt.int8, kind="ExternalInput")
    wt = nc.dram_tensor("wt", [P, 9 * COUT], mybir.dt.bfloat16, kind="ExternalInput")
    tb = nc.dram_tensor("tb", [P, G], mybir.dt.float32, kind="ExternalInput")
    lo = nc.dram_tensor("lo", [P, G], mybir.dt.float32, kind="ExternalInput")
    hi = nc.dram_tensor("hi", [P, G], mybir.dt.float32, kind="ExternalInput")
    ys = nc.dram_tensor("ys", [B_LOC, COUT, H, W], mybir.dt.uint8, kind="ExternalOutput")

    with tile.TileContext(nc) as tc, ExitStack() as ctx:
        wpool = ctx.enter_context(tc.tile_pool(name="wpool", bufs=1))
        cpool = ctx.enter_context(tc.tile_pool(name="cpool", bufs=1))
        xspool = ctx.enter_context(tc.tile_pool(name="xspool", bufs=3))
        xppool = ctx.enter_context(tc.tile_pool(name="xppool", bufs=1))
        pspool = ctx.enter_context(tc.tile_pool(name="pspool", bufs=6, space="PSUM"))
        i1pool = ctx.enter_context(tc.tile_pool(name="i1pool", bufs=2))
        i2pool = ctx.enter_context(tc.tile_pool(name="i2pool", bufs=2))
        opool = ctx.enter_context(tc.tile_pool(name="opool", bufs=3))

        wt_s = wpool.tile([P, 9 * COUT], mybir.dt.bfloat16)
        tb_s = cpool.tile([P, G], mybir.dt.float32)
        lo_s = cpool.tile([P, G], mybir.dt.float32)
        hi_s = cpool.tile([P, G], mybir.dt.float32)
        xpad = xppool.tile([P, NB], mybir.dt.bfloat16)

        # HAM pre-warm: the PE is idle ~7-11us at start while input DMAs run.
        # A stream of zero matmuls during that window gets the clock gate to
        # K=8/8 (2.4 GHz) before the first real matmul; no data deps.
        zpool = ctx.enter_context(tc.tile_pool(name="zpool", bufs=1))
        wupool = ctx.enter_context(tc.tile_pool(name="wupool", bufs=1,
                                                space="PSUM"))
        zs = zpool.tile([P, 384], mybir.dt.bfloat16)
        nc.vector.memset(zs[:], 0.0)
        wps = wupool.tile([P, 384], mybir.dt.float32)
        for _ in range(12):
            nc.tensor.matmul(wps[:], lhsT=zs[:, :P], rhs=zs[:],
                             start=True, stop=True)
        nc.vector.memset(xpad[:, 0:1], 0.0)
        nc.vector.memset(xpad[:, NB - 1:NB], 0.0)
        inners = []
        for img in range(B_LOC):
            base = 1 + img * IMG
            # pad row 0 (+ row 1 col 0)
            nc.vector.memset(xpad[:, base:base + Wp + 1], 0.0)
            # pad row Hp-1
            nc.vector.memset(xpad[:, base + (Hp - 1) * Wp: base + IMG], 0.0)
            inner = xpad[:, base + Wp: base + Wp + H * Wp].rearrange(
                "p (h w) -> p h w", w=Wp)
            nc.vector.memset(inner[:, :, Wp - 1:Wp], 0.0)   # col Wp-1, rows 1..H
            nc.vector.memset(inner[:, 1:, 0:1], 0.0)        # col 0, rows 2..H
            inners.append(inner)

        # Sub-image load/cast granularity so the first matmuls start sooner;
        # critical-path deps (x rows 0.., then tap0/g0 weights) issue first.
        def load_rows(img, r0, nrows):
            xstage = xspool.tile([P, (H // 2) * W], mybir.dt.int8, name="xstage")
            st = xstage[:, :nrows * W]
            nc.sync.dma_start(
                st, xs.ap()[img, :, r0:r0 + nrows, :].rearrange("c h w -> c (h w)"))
            # cast int8 -> bf16 into padded interior (exact: |x| <= 256)
            nc.scalar.copy(inners[img][:, r0:r0 + nrows, 1:1 + W],
                           st.rearrange("p (h w) -> p h w", w=W))

        QR = H // 4
        load_rows(0, 0, QR)
        nc.sync.dma_start(wt_s[:, :P], wt.ap()[:, :P])            # tap0, g=0
        nc.sync.dma_start(wt_s[:, P:9 * P], wt.ap()[:, P:9 * P])  # taps 1-8, g=0
        load_rows(0, QR, QR)
        nc.sync.dma_start(tb_s[:], tb.ap())
        nc.sync.dma_start(lo_s[:], lo.ap())
        nc.sync.dma_start(hi_s[:], hi.ap())
        load_rows(0, 2 * QR, QR)
        load_rows(0, 3 * QR, QR)
        load_rows(1, 0, QR)                                       # needed ~25us
        load_rows(1, QR, QR)
        nc.sync.dma_start(wt_s[:, 9 * P:], wt.ap()[:, 9 * P:])    # g=1, needed ~38us
        load_rows(1, 2 * QR, QR)
        load_rows(1, 3 * QR, QR)
        for img in range(2, B_LOC):
            load_rows(img, 0, H // 2)
            load_rows(img, H // 2, H // 2)

        for img in range(B_LOC):
            for g in range(G):
                base = 1 + img * IMG
                for ch in range(N_CH):
                    hp0 = 1 + ch * ROWS_PER_CH
                    psumt = pspool.tile([P, NFREE], mybir.dt.float32)
                    for tap in range(9):
                        dh, dw = tap // 3, tap % 3
                        rs = base + (hp0 + dh - 1) * Wp + (dw - 1)
                        nc.tensor.matmul(
                            psumt[:],
                            lhsT=wt_s[:, (g * 9 + tap) * P:(g * 9 + tap + 1) * P],
                            rhs=xpad[:, rs:rs + NFREE],
                            start=(tap == 0),
                            stop=(tap == 8),
                        )
                    # Last chunk: split the requant chain in two so the
                    # post-matmul tail is shorter; otherwise one pass.
                    last = (img == B_LOC - 1 and g == G - 1 and ch == N_CH - 1)
                    for r0, rn in ([(0, 4), (4, 4)] if last
                                   else [(0, ROWS_PER_CH)]):
                        cs, cn = r0 * Wp, rn * Wp
                        it = i1pool.tile([P, NFREE], mybir.dt.int32,
                                         name="it")[:, :cn]
                        nc.scalar.activation(it, psumt[:, cs:cs + cn],
                                             mybir.ActivationFunctionType.Identity,
                                             bias=tb_s[:, g:g + 1], scale=1.0)
                        ct = i2pool.tile([P, NFREE], mybir.dt.int32,
                                         name="ct")[:, :cn]
                        nc.vector.tensor_scalar(ct, it, shift, None,
                                                mybir.AluOpType.arith_shift_right)
                        # clamp + compact pad columns: strided read, tight write
                        ot = opool.tile([P, ROWS_PER_CH * W], mybir.dt.uint8,
                                        name="ot")[:, :rn * W]
                        csrc = ct.rearrange("p (r w) -> p r w", w=Wp)[:, :, 1:1 + W]
                        nc.vector.tensor_scalar(
                            ot.rearrange("p (r w) -> p r w", w=W), csrc,
                            lo_s[:, g:g + 1], hi_s[:, g:g + 1],
                            mybir.AluOpType.max, mybir.AluOpType.min)
                        # contiguous per partition on both sides
                        nc.sync.dma_start(
                            ys.ap()[img, g * P:(g + 1) * P,
                                    hp0 - 1 + r0:hp0 - 1 + r0 + rn, :]
                            .rearrange("c h w -> c (h w)"),
                            ot)

    nc.compile()
    return nc


def _pack_inputs(x, weight, t, n, act_min, act_max):
    x = np.asarray(x)
    weight = np.asarray(weight)
    t = np.asarray(t).reshape(COUT)
    n = np.asarray(n).reshape(COUT)
    act_min = np.asarray(act_min).reshape(COUT)
    act_max = np.asarray(act_max).reshape(COUT)

    assert x.shape == (B, P, H, W) and weight.shape == (COUT, P, 3, 3)
    nval = int(n[0])
    assert np.all(n == nval) and nval <= 0, "non-uniform/positive BN shift unsupported"
    shift = -nval
    assert np.all(act_min >= 0) and np.all(act_max <= 255), \
        "act range must fit uint8 (pure_positive path)"
    # int8 shipping + bf16/fp32 exactness preconditions
    assert x.min() >= -128 and x.max() <= 127
    assert np.abs(weight).max(initial=0) <= 256

    # lhsT pack, g-major: wt[ci, (g*9+tap)*P + co] = weight[g*P+co, ci, kh, kw]
    wr = weight.reshape(G, P, P, 9)            # [g, co, ci, tap]
    wr = wr.transpose(2, 0, 3, 1)              # [ci, g, tap, co]
    wt_np = np.ascontiguousarray(wr.reshape(P, 9 * COUT)).astype(ml_dtypes.bfloat16)

    tb_np = np.ascontiguousarray(t.reshape(G, P).T).astype(np.float32)
    lo_np = np.ascontiguousarray(act_min.reshape(G, P).T).astype(np.float32)
    hi_np = np.ascontiguousarray(act_max.reshape(G, P).T).astype(np.float32)
    return x, wt_np, tb_np, lo_np, hi_np, shift


def kernel(x, weight, t, n, act_min, act_max):
    x, wt_np, tb_np, lo_np, hi_np, shift = _pack_inputs(
        x, weight, t, n, act_min, act_max)

    if shift not in _cache:
        _cache[shift] = _build(shift)
    nc = _cache[shift]

    x8 = x.astype(np.int8)  # exact: setup guarantees int8-valued data
    in_maps = []
    for c in range(N_CORES):
        in_maps.append({
            "xs": np.ascontiguousarray(x8[c * B_LOC:(c + 1) * B_LOC]),
            "wt": wt_np,
            "tb": tb_np,
            "lo": lo_np,
            "hi": hi_np,
        })
    res = run_bass_kernel_spmd(nc, in_maps, core_ids=list(range(N_CORES)))
    out = np.concatenate([res.results[c]["ys"] for c in range(N_CORES)], axis=0)
    return out


# revision 28
# speedup vs baseline: 1.0328x; 1.0055x over previous
"""Trainium2 Bass kernel for int8-valued Conv2d(128->256, 3x3, pad 1) + BN-add +
shift requant + clip + uint8 cast, over x[32,128,56,56].

Strategy: data-parallel over batch across 8 NeuronCores (4 images/core).
Per core, the conv runs as 9 PSUM-accumulated bf16 matmuls (one per 3x3 tap)
with Cin=128 on the partition axis. int8-valued data is exact in bf16, and all
fp32 accumulations stay below 2^24, so the matmul path is integer-exact.
Input is laid out zero-padded to 58x58 in SBUF so each tap's rhs is a plain
column-shifted window. Output rows are produced in chunks of 8 rows
(N = 8*58 = 464 <= 512 fp32 PSUM bank columns).

Requant (matches reference op order): ACT adds per-channel t (fp32, integral,
out int32), DVE arithmetic-shifts right by s (int32->int32; bitwise ops cannot
cast), DVE clamps to [act_min, act_max] (fp32 ALU pair, writes uint8). Every
dtype conversion happens on integral values, so rounding-mode differences
between sim and HW are moot.
"""

import numpy as np
import ml_dtypes
from contextlib import ExitStack

import concourse.bass as bass  # noqa: F401  (registers engine types)
import concourse.mybir as mybir
import concourse.tile as tile
from concourse import bacc
from concourse.bass_utils import run_bass_kernel_spmd

# Problem constants (hardcoded per contract)
N_CORES = 8
B = 32
B_LOC = B // N_CORES          # 4 images per core
P = 128                       # Cin = partition dim
H = W = 56
Hp = Wp = 58                  # padded
IMG = Hp * Wp                 # 3364 padded pixels per image
NB = B_LOC * IMG + 2          # + leading/trailing guard column
COUT = 256
G = COUT // P                 # 2 Cout halves
ROWS_PER_CH = 8
N_CH = H // ROWS_PER_CH       # 7 chunks per image
NFREE = ROWS_PER_CH * Wp      # 464 columns per matmul

_cache = {}


def _build(shift: int):
    """Build + compile the per-core Bass program. Same NEFF on all 8 cores."""
    nc = bacc.Bacc("TRN2", target_bir_lowering=False, debug=False,
                   num_devices=N_CORES)

    xs = nc.dram_tensor("xs", [B_LOC, P, H, W], mybir.dt.int8, kind="ExternalInput")
    wt = nc.dram_tensor("wt", [P, 9 * COUT], mybir.dt.bfloat16, kind="ExternalInput")
    tb = nc.dram_tensor("tb", [P, G], mybir.dt.float32, kind="ExternalInput")
    lo = nc.dram_tensor("lo", [P, G], mybir.dt.float32, kind="ExternalInput")
    hi = nc.dram_tensor("hi", [P, G], mybir.dt.float32, kind="ExternalInput")
    ys = nc.dram_tensor("ys", [B_LOC, COUT, H, W], mybir.dt.uint8, kind="ExternalOutput")

    with tile.TileContext(nc) as tc, ExitStack() as ctx:
        wpool = ctx.enter_context(tc.tile_pool(name="wpool", bufs=1))
        cpool = ctx.enter_context(tc.tile_pool(name="cpool", bufs=1))
        xspool = ctx.enter_context(tc.tile_pool(name="xspool", bufs=3))
        xppool = ctx.enter_context(tc.tile_pool(name="xppool", bufs=1))
        pspool = ctx.enter_context(tc.tile_pool(name="pspool", bufs=7, space="PSUM"))
        i1pool = ctx.enter_context(tc.tile_pool(name="i1pool", bufs=3))
        i2pool = ctx.enter_context(tc.tile_pool(name="i2pool", bufs=3))
        opool = ctx.enter_context(tc.tile_pool(name="opool", bufs=4))

        wt_s = wpool.tile([P, 9 * COUT], mybir.dt.bfloat16)
        tb_s = cpool.tile([P, G], mybir.dt.float32)
        lo_s = cpool.tile([P, G], mybir.dt.float32)
        hi_s = cpool.tile([P, G], mybir.dt.float32)
        xpad = xppool.tile([P, NB], mybir.dt.bfloat16)

        # HAM pre-warm: the PE is idle ~7-11us at start while input DMAs run.
        # A stream of zero matmuls during that window gets the clock gate to
        # K=8/8 (2.4 GHz) before the first real matmul; no data deps.
        zpool = ctx.enter_context(tc.tile_pool(name="zpool", bufs=1))
        wupool = ctx.enter_context(tc.tile_pool(name="wupool", bufs=1,
                                                space="PSUM"))
        zs = zpool.tile([P, 384], mybir.dt.bfloat16)
        nc.vector.memset(zs[:], 0.0)
        wps = wupool.tile([P, 384], mybir.dt.float32)
        for _ in range(12):
            nc.tensor.matmul(wps[:], lhsT=zs[:, :P], rhs=zs[:],
                             start=True, stop=True)
        nc.vector.memset(xpad[:, 0:1], 0.0)
        nc.vector.memset(xpad[:, NB - 1:NB], 0.0)
        inners = []
        for img in range(B_LOC):
            base = 1 + img * IMG
            # pad row 0 (+ row 1 col 0)
            nc.vector.memset(xpad[:, base:base + Wp + 1], 0.0)
            # pad row Hp-1
            nc.vector.memset(xpad[:, base + (Hp - 1) * Wp: base + IMG], 0.0)
            inner = xpad[:, base + Wp: base + Wp + H * Wp].rearrange(
                "p (h w) -> p h w", w=Wp)
            nc.vector.memset(inner[:, :, Wp - 1:Wp], 0.0)   # col Wp-1, rows 1..H
            nc.vector.memset(inner[:, 1:, 0:1], 0.0)        # col 0, rows 2..H
            inners.append(inner)

        # Sub-image load/cast granularity so the first matmuls start sooner;
        # critical-path deps (x rows 0.., then tap0/g0 weights) issue first.
        def load_rows(img, r0, nrows):
            xstage = xspool.tile([P, (H // 2) * W], mybir.dt.int8, name="xstage")
            st = xstage[:, :nrows * W]
            nc.sync.dma_start(
                st, xs.ap()[img, :, r0:r0 + nrows, :].rearrange("c h w -> c (h w)"))
            # cast int8 -> bf16 into padded interior (exact: |x| <= 256)
            nc.scalar.copy(inners[img][:, r0:r0 + nrows, 1:1 + W],
                           st.rearrange("p (h w) -> p h w", w=W))

        QR = H // 4
        load_rows(0, 0, QR)
        nc.sync.dma_start(wt_s[:, :P], wt.ap()[:, :P])            # tap0, g=0
        nc.sync.dma_start(wt_s[:, P:9 * P], wt.ap()[:, P:9 * P])  # taps 1-8, g=0
        load_rows(0, QR, QR)
        nc.sync.dma_start(tb_s[:], tb.ap())
        nc.sync.dma_start(lo_s[:], lo.ap())
        nc.sync.dma_start(hi_s[:], hi.ap())
        load_rows(0, 2 * QR, QR)
        load_rows(0, 3 * QR, QR)
        load_rows(1, 0, QR)                                       # needed ~25us
        load_rows(1, QR, QR)
        nc.sync.dma_start(wt_s[:, 9 * P:], wt.ap()[:, 9 * P:])    # g=1, needed ~38us
        load_rows(1, 2 * QR, QR)
        load_rows(1, 3 * QR, QR)
        for img in range(2, B_LOC):
            load_rows(img, 0, H // 2)
            load_rows(img, H // 2, H // 2)

        for img in range(B_LOC):
            for g in range(G):
                base = 1 + img * IMG
                for ch in range(N_CH):
                    hp0 = 1 + ch * ROWS_PER_CH
                    psumt = pspool.tile([P, NFREE], mybir.dt.float32)
                    for tap in range(9):
                        dh, dw = tap // 3, tap % 3
                        rs = base + (hp0 + dh - 1) * Wp + (dw - 1)
                        nc.tensor.matmul(
                            psumt[:],
                            lhsT=wt_s[:, (g * 9 + tap) * P:(g * 9 + tap + 1) * P],
                            rhs=xpad[:, rs:rs + NFREE],
                            start=(tap == 0),
                            stop=(tap == 8),
                        )
                    # Last chunk: split the requant chain in two so the
                    # post-matmul tail is shorter; otherwise one pass.
                    last = (img == B_LOC - 1 and g == G - 1 and ch == N_CH - 1)
                    for r0, rn in ([(0, 4), (4, 4)] if last
                                   else [(0, ROWS_PER_CH)]):
                        cs, cn = r0 * Wp, rn * Wp
                        it = i1pool.tile([P, NFREE], mybir.dt.int32,
                                         name="it")[:, :cn]
                        nc.scalar.activation(it, psumt[:, cs:cs + cn],
                                             mybir.ActivationFunctionType.Identity,
                                             bias=tb_s[:, g:g + 1], scale=1.0)
                        ct = i2pool.tile([P, NFREE], mybir.dt.int32,
                                         name="ct")[:, :cn]
                        nc.vector.tensor_scalar(ct, it, shift, None,
                                                mybir.AluOpType.arith_shift_right)
                        # clamp + compact pad columns: strided read, tight write
                        ot = opool.tile([P, ROWS_PER_CH * W], mybir.dt.uint8,
                                        name="ot")[:, :rn * W]
                        csrc = ct.rearrange("p (r w) -> p r w", w=Wp)[:, :, 1:1 + W]
                        nc.vector.tensor_scalar(
                            ot.rearrange("p (r w) -> p r w", w=W), csrc,
                            lo_s[:, g:g + 1], hi_s[:, g:g + 1],
                            mybir.AluOpType.max, mybir.AluOpType.min)
                        # contiguous per partition on both sides
                        nc.sync.dma_start(
                            ys.ap()[img, g * P:(g + 1) * P,
                                    hp0 - 1 + r0:hp0 - 1 + r0 + rn, :]
                            .rearrange("c h w -> c (h w)"),
                            ot)

    nc.compile()
    return nc


def _pack_inputs(x, weight, t, n, act_min, act_max):
    x = np.asarray(x)
    weight = np.asarray(weight)
    t = np.asarray(t).reshape(COUT)
    n = np.asarray(n).reshape(COUT)
    act_min = np.asarray(act_min).reshape(COUT)
    act_max = np.asarray(act_max).reshape(COUT)

    assert x.shape == (B, P, H, W) and weight.shape == (COUT, P, 3, 3)
    nval = int(n[0])
    assert np.all(n == nval) and nval <= 0, "non-uniform/positive BN shift unsupported"
    shift = -nval
    assert np.all(act_min >= 0) and np.all(act_max <= 255), \
        "act range must fit uint8 (pure_positive path)"
    # int8 shipping + bf16/fp32 exactness preconditions
    assert x.min() >= -128 and x.max() <= 127
    assert np.abs(weight).max(initial=0) <= 256

    # lhsT pack, g-major: wt[ci, (g*9+tap)*P + co] = weight[g*P+co, ci, kh, kw]
    wr = weight.reshape(G, P, P, 9)            # [g, co, ci, tap]
    wr = wr.transpose(2, 0, 3, 1)              # [ci, g, tap, co]
    wt_np = np.ascontiguousarray(wr.reshape(P, 9 * COUT)).astype(ml_dtypes.bfloat16)

    tb_np = np.ascontiguousarray(t.reshape(G, P).T).astype(np.float32)
    lo_np = np.ascontiguousarray(act_min.reshape(G, P).T).astype(np.float32)
    hi_np = np.ascontiguousarray(act_max.reshape(G, P).T).astype(np.float32)
    return x, wt_np, tb_np, lo_np, hi_np, shift


def kernel(x, weight, t, n, act_min, act_max):
    x, wt_np, tb_np, lo_np, hi_np, shift = _pack_inputs(
        x, weight, t, n, act_min, act_max)

    if shift not in _cache:
        _cache[shift] = _build(shift)
    nc = _cache[shift]

    x8 = x.astype(np.int8)  # exact: setup guarantees int8-valued data
    in_maps = []
    for c in range(N_CORES):
        in_maps.append({
            "xs": np.ascontiguousarray(x8[c * B_LOC:(c + 1) * B_LOC]),
            "wt": wt_np,
            "tb": tb_np,
            "lo": lo_np,
            "hi": hi_np,
        })
    res = run_bass_kernel_spmd(nc, in_maps, core_ids=list(range(N_CORES)))
    out = np.concatenate([res.results[c]["ys"] for c in range(N_CORES)], axis=0)
    return out
